# revision 88
# baseline (speedup 1.0000x reference)
"""Group-causal sliding-window attention on 8 Trainium2 NeuronCores.

Reference semantics (B=2, H=8, N=2048, D=64, group_size=16, window=256):
  allowed(q, k) = (k//16 <= q//16) and (k >= q - 256) and key_padding[b, k]
  out = softmax(q @ k.T / 8 + bias) @ v

Sharding: 16 (b, h) pairs -> 2 per core (batch+head parallelism), no
cross-device comms.

Per-core device kernel (bf16, ~21.2us vs 29.4us fp32r baseline):
  Queries processed in tiles of 256. For query tile t the allowed keys live
  in 128-key blocks kt = 2t-2 .. 2t+1, but block 2t-2 (j0) only reaches the
  first 128 queries and block 2t+1 (j3) only the last 128 (window/group
  cuts), so scores are computed TRANSPOSED in bf16 (1 cycle/row at any
  width) over exactly the live 768 columns: st layout per 2-bank PSUM tile
  is [j3h(128) | j1(256) | dead(128) | j2(256) | j0h(128) | dead(128)],
  keeping every matmul inside a 2KB PSUM bank.

  Group-causal staircases on the diagonal blocks are folded into the score
  matmul via 16 extra contraction rows (rank-8 -BIG decompositions split by
  block parity); the sliding-window triangle on j1's second half and j0h is
  a static 0/1 band multiplied into exp(S) as ONE strided DVE op per tile.
  exp runs on the scalar engine over the live cols ([p, 2, 384] AP, scale
  1/8 folded in, bf16 out) and is the bottleneck engine (~12.6us busy, the
  3-cols-per-query floor); the whole schedule exists to keep it saturated.
  Row sums come free from P@V via a ones-column appended to V. P@V
  accumulates into the DEAD 128-col regions of the same score tile (as two
  [65,128] groups per tile), so the 4x 2-bank score tiles exactly fill all
  8 PSUM banks. A DVE copy gathers the two groups into a bf16 SBUF buffer
  DMA'd out raw (unnormalized O^T plus row sums); the host divides and
  transposes.

  Schedule: jobs run t=0..7 per head (light t=0/t=1 first — they need only
  kqa[0:1024) so the ACT train starts right after one small DMA); K and Q
  interleave in one DRAM tensor so each load window is a single contiguous
  DMA; head 0 loads via sync/HWDGE, head 1 via gpsimd/SWDGE in parallel.
  pv/copy of job i-3 is emitted BEFORE scores of job i so the copy precedes
  the next band mul in the DVE queue (PSUM slot-free chain). Outputs leave
  per 256-col half as soon as a pair completes, on engines chosen to keep
  the final t=7 chains on free queues. A memset-fed dummy exp + matmul at
  t~0 pulls the ACT table load and the PE p-state ramp clock into the
  initial DMA shadow.
"""

import sys

sys.path.insert(0, "/opt/trn_rl_repo")

from contextlib import ExitStack

import ml_dtypes
import numpy as np

import concourse.bacc as bacc
import concourse.tile as tile
from concourse import mybir
from concourse.bass_utils import run_bass_kernel_spmd

B, H, N, D = 2, 8, 2048, 64
G = 16          # group size
WIN = 256       # sliding window
NCORES = 8
HPC = 2         # (b, h) pairs per core
NB = N // 128   # 16 key blocks per head
NT = N // 256   # 8 query tiles of 256 per head
BIG = 1e30
F32 = mybir.dt.float32
BF16 = mybir.dt.bfloat16
EXP = mybir.ActivationFunctionType.Exp


def _host_masks():
    """Static fold-row / band patterns shared by all cores."""
    i = np.arange(N)
    mod = i % 256
    qlg1 = mod // 16            # local group id, first half of a 256-tile
    qlg2 = (mod - 128) // 16    # local group id, second half
    g = np.arange(8)[:, None]
    # q-side fold indicator rows [16, N]: rows 0:8 = b2 (2nd-half queries),
    # rows 8:16 = b1 (1st-half queries)
    b1 = ((mod < 128) & (qlg1 == g)).astype(np.float32)
    b2 = ((mod >= 128) & (qlg2 == g)).astype(np.float32)
    qrows = np.concatenate([b2, b1], axis=0)

    kt = i // 128
    klg = (i % 128) // 16
    even = kt % 2 == 0
    # k-side fold rows [16, N]: -BIG * [klg > g], split by block parity.
    # Row r pairs with qrows row r: (b2,a2) then (b1,a1).
    a1 = np.where(even[None, :] & (klg[None, :] > g), -BIG, 0.0).astype(np.float32)
    a2 = np.where(~even[None, :] & (klg[None, :] > g), -BIG, 0.0).astype(np.float32)
    krows = np.concatenate([a2, a1], axis=0)

    # Window band (multiplicative on E): allowed iff kl >= ql. Two copies
    # side by side so one strided mul covers j1-2nd-half and j0h.
    kl = np.arange(128)[:, None]
    ql = np.arange(128)[None, :]
    band = np.where(kl < ql, 0.0, 1.0).astype(np.float32)
    band2 = np.concatenate([band, band], axis=1)  # [128, 256]
    return qrows, krows, band2


def _build_module():
    nc = bacc.Bacc("TRN2", target_bir_lowering=False, debug=False)
    # kqa interleaves K and Q along time so one contiguous DMA covers a
    # window of jobs: per head, u-group u = [K blk 2u | K blk 2u+1 | Q tile u]
    kqa_d = nc.dram_tensor("kqa", [80, HPC * 2 * N], BF16, kind="ExternalInput")
    v_d = nc.dram_tensor("vp", [128, HPC * NB * 65], BF16, kind="ExternalInput")
    band_d = nc.dram_tensor("band2", [128, 256], BF16, kind="ExternalInput")
    # output: per head, unnormalized O^T (rows 0:64) + row sums (row 64),
    # column q = global query index. Rows 65:127 are scratch padding for the
    # kv_writeback store path (d_head must be a multiple of 128).
    o_d = nc.dram_tensor("o", [HPC, 128, N], BF16, kind="ExternalOutput")

    def mm(out, lhsT, rhs, **kw):
        nc.tensor.matmul(out, lhsT, rhs, **kw)

    with tile.TileContext(nc) as tc, ExitStack() as ctx:
        const = ctx.enter_context(tc.tile_pool(name="const", bufs=1))
        kqa = const.tile([80, HPC * 2 * N], BF16)
        vp = const.tile([128, HPC * NB * 65], BF16)
        band2 = const.tile([128, 256], BF16)


        sp = ctx.enter_context(tc.tile_pool(name="scores", bufs=4, space="PSUM"))
        ep = ctx.enter_context(tc.tile_pool(name="expdat", bufs=5))
        o12 = ctx.enter_context(tc.tile_pool(name="ob12", bufs=2))
        osp = ctx.enter_context(tc.tile_pool(name="obs", bufs=4))

        # Warm-up during the initial DMA shadow: ACT exp-table load and the
        # PE p-state ramp clock, fed from a memset so no DMA is needed.
        warm = ep.tile([4, 8], BF16, tag="warm")
        nc.vector.memset(warm[:], 0.25)
        nc.scalar.activation(warm[0:1, 0:2], warm[0:1, 4:6], EXP)
        wps = sp.tile([128, 1024], F32, tag="st", name="wps")
        mm(wps[0:8, 0:8], warm[:, 0:8], warm[:, 0:8], start=True, stop=True)

        # Input loads, staged in job-need order (jobs run t=0,1,...,7). Head 0
        # (+ band) flows through HWDGE via sync; head 1 through SWDGE via
        # gpsimd so the two descriptor-gen paths run in parallel. u-group
        # ranges: t=0,1 need [0:1024), t=2,3 [512:2560), t>=4 [2560:4096).
        # vp is split so the first 4 V-blocks (needed by t=0,1 P@V) land fast.
        HB = 2 * N  # per-head kqa columns

        def ld(eng, a, b):
            eng.dma_start(kqa[:, a:b], kqa_d.ap()[:, a:b])

        def ldv(eng, hp, a, b):
            o = hp * NB * 65
            eng.dma_start(vp[:, o + a:o + b], v_d.ap()[:, o + a:o + b])

        ld(nc.sync, 0, 1024)
        ld(nc.gpsimd, HB, HB + 1024)
        ldv(nc.sync, 0, 0, 4 * 65)
        ldv(nc.gpsimd, 1, 0, 4 * 65)
        nc.sync.dma_start(band2[:], band_d.ap())
        ld(nc.sync, 1024, 2560)
        ld(nc.gpsimd, HB + 1024, HB + 2560)
        ldv(nc.sync, 0, 4 * 65, NB * 65)
        ldv(nc.gpsimd, 1, 4 * 65, NB * 65)
        ld(nc.sync, 2560, 4096)
        ld(nc.gpsimd, HB + 2560, HB + 4096)

        band2r = band2[:].rearrange("p (a c) -> p a c", c=128)

        def stage_scores(t, hp, split_exp=False):
            qb = hp * HB + 512 * t + 256
            kb = lambda j: (lambda m: hp * HB + 512 * (m // 2) + 128 * (m % 2))(
                2 * t - 2 + j
            )
            st = sp.tile([128, 1024], F32, tag="st", name="st")
            et = ep.tile([128, 768], BF16, tag="etb", name="et")
            if t == 0:
                # t=0 has only j2 + j3h; park j3h's scores at [384:512] so
                # one contiguous exp covers st[384:768] -> et[0:384]
                # (et: j3h at [0:128], j2 at [128:384]); P@V groups then use
                # st[0:128] / st[896:1024] as their dead-column outputs.
                mm(st[:, 512:768], kqa[0:80, kb(2):kb(2) + 128],
                   kqa[0:80, qb:qb + 256], start=True, stop=True)
                mm(st[:, 384:512], kqa[0:80, kb(3):kb(3) + 128],
                   kqa[0:80, qb + 128:qb + 256], start=True, stop=True)
                nc.scalar.activation(et[:, 0:384], st[:, 384:768], EXP,
                                     scale=D ** -0.5)
                return st, et
            mm(st[:, 0:128], kqa[0:80, kb(3):kb(3) + 128],
               kqa[0:80, qb + 128:qb + 256], start=True, stop=True)
            mm(st[:, 128:384], kqa[0:64, kb(1):kb(1) + 128],
               kqa[0:64, qb:qb + 256], start=True, stop=True)
            if split_exp:
                # first job: exp the first chunk as soon as its two matmuls
                # land so the ACT train starts ~0.7us earlier
                nc.scalar.activation(et[:, 0:384], st[:, 0:384], EXP,
                                     scale=D ** -0.5)
            mm(st[:, 512:768], kqa[0:80, kb(2):kb(2) + 128],
               kqa[0:80, qb:qb + 256], start=True, stop=True)
            mm(st[:, 768:896], kqa[0:64, kb(0):kb(0) + 128],
               kqa[0:64, qb:qb + 128], start=True, stop=True)
            if split_exp:
                nc.scalar.activation(et[:, 384:768], st[:, 512:896], EXP,
                                     scale=D ** -0.5)
            else:
                sin = st[:].rearrange("p (a c) -> p a c", c=512)[:, :, 0:384]
                eout = et[:].rearrange("p (a c) -> p a c", c=384)
                nc.scalar.activation(eout, sin, EXP, scale=D ** -0.5)
            # window band on j1's second half (cols 256:384) and j0h
            # (cols 640:768): one strided DVE mul
            bsl = et[:, 256:768].rearrange("p (a c) -> p a c", c=128)[:, 0::3]
            nc.vector.tensor_mul(bsl, bsl, band2r)
            return st, et

        # osb buffers: pairs (1,2) of each head share a [65,1024] buffer
        # stored in one DMA; pairs 0 and 3 get [65,512] buffers stored per
        # 256-col half (pair 0 early, pair 3's t=7 half is the tail).
        osb_map = {}
        ndone = {}

        def osb_slot(hp, p):
            if p in (1, 2):
                key = (hp, 12)
                if key not in osb_map:
                    osb_map[key] = o12.tile([65, 1024], BF16, name="ob12")
                return osb_map[key], (p - 1) * 512
            key = (hp, p)
            if key not in osb_map:
                osb_map[key] = osp.tile([65, 512], BF16, name="obs")
            return osb_map[key], 0

        def stage_pv(t, hp, st, et):
            vb = lambda kt: (hp * NB + kt) * 65
            A = st[0:65, 384:512]
            Bv = st[0:65, 896:1024]
            if t == 0:
                A = st[0:65, 0:128]
                mm(A, vp[:, vb(0):vb(0) + 65], et[:, 128:256],
                   start=True, stop=True)
                mm(Bv, vp[:, vb(0):vb(0) + 65], et[:, 256:384],
                   start=True, stop=False)
                mm(Bv, vp[:, vb(1):vb(1) + 65], et[:, 0:128],
                   start=False, stop=True)
            else:
                k0, k1, k2, k3 = 2 * t - 2, 2 * t - 1, 2 * t, 2 * t + 1
                mm(A, vp[:, vb(k1):vb(k1) + 65], et[:, 128:256],
                   start=True, stop=False)
                mm(Bv, vp[:, vb(k2):vb(k2) + 65], et[:, 512:640],
                   start=True, stop=False)
                mm(A, vp[:, vb(k2):vb(k2) + 65], et[:, 384:512],
                   start=False, stop=False)
                mm(Bv, vp[:, vb(k3):vb(k3) + 65], et[:, 0:128],
                   start=False, stop=False)
                # band-masked inputs last
                mm(A, vp[:, vb(k0):vb(k0) + 65], et[:, 640:768],
                   start=False, stop=True)
                mm(Bv, vp[:, vb(k1):vb(k1) + 65], et[:, 256:384],
                   start=False, stop=True)
            # gather the two [65,128] groups into the bf16 out buffer
            p = t // 2
            ob, base = osb_slot(hp, p)
            base += (t % 2) * 256
            stride = 7 if t == 0 else 4
            off = 0 if t == 0 else 384
            src = st[0:65, off:1024].rearrange(
                "p (a c) -> p a c", c=128
            )[:, 0::stride]
            dst = ob[0:65, base:base + 256].rearrange("p (a c) -> p a c", c=128)
            if (t, hp) == (7, 0):
                # the ACT engine is idle once its last exp is out; doing
                # (7,0)'s PSUM->SBUF copy there unclogs the DVE queue for
                # the final job's band mul and copy (the critical chain)
                nc.scalar.activation(dst, src, mybir.ActivationFunctionType.Copy)
            else:
                nc.vector.tensor_copy(dst, src)
            if p == 0 or p == 3:
                # store this half right away; early h1 halves ride the
                # SWDGE/gpsimd path to keep HWDGE free. At the end, (7,0)
                # leaves via the idle SWDGE/Pool queue so the very last
                # store (7,1) gets sync's faster DGE path with HWDGE free.
                if t == 7:
                    eng = nc.gpsimd if hp == 0 else nc.sync
                elif hp == 0 or t == 6:
                    eng = nc.sync
                else:
                    eng = nc.gpsimd
                c0 = 512 * p + 256 * (t % 2)
                eng.dma_start(o_d.ap()[hp, 0:65, c0:c0 + 256],
                              osb_map[(hp, p)][0:65, base:base + 256])
                return
            # store when the (1,2) pair batch is complete
            bkey = (hp, 12)
            ndone[bkey] = ndone.get(bkey, 0) + 1
            if ndone[bkey] == 4:
                nc.sync.dma_start(o_d.ap()[hp, 0:65, 512:1536],
                                  osb_map[bkey][:])
                del osb_map[bkey]

        # software pipeline: scores lead P@V by PVLAG jobs. Light t=0/t=1
        # jobs FIRST: they need only kqa[0:1024) so the ACT train starts as
        # soon as one small DMA lands.
        jobs = [(t, hp) for t in (0, 1, 2, 3, 4, 5, 6, 7) for hp in range(HPC)]
        from collections import deque

        # pv/copy of job i-PVLAG is emitted BEFORE scores of job i: the copy
        # must precede the next band mul in the DVE queue, else the PSUM
        # slot-free chain runs through band(i+2) -> exp(i+2) and stalls ACT
        PVLAG = 2
        pq = deque()
        for i, (t, hp) in enumerate(jobs):
            if len(pq) >= PVLAG:
                stage_pv(*pq.popleft())
            pq.append((t, hp, *stage_scores(t, hp)))
        while pq:
            stage_pv(*pq.popleft())

    nc.compile()
    return nc


_NC = None


def _get_module():
    global _NC
    if _NC is None:
        _NC = _build_module()
    return _NC


def _host_prep(q, k, v):
    """Build per-core input maps."""
    qrows, krows, band2 = _host_masks()
    bf = ml_dtypes.bfloat16
    ones = np.ones((NB, 128, 1), dtype=np.float32)
    in_maps = []
    for c in range(NCORES):
        kqa_, vp_ = [], []
        for hp in range(HPC):
            bh = HPC * c + hp
            b, h = bh // H, bh % H
            qa = np.concatenate([q[b, h].T, qrows], axis=0)  # [80, N]
            ka = np.concatenate([k[b, h].T, krows], axis=0)  # [80, N]
            # interleave: u-group u = [K blk 2u | K blk 2u+1 | Q tile u]
            kau = ka.reshape(80, NT, 256)
            qau = qa.reshape(80, NT, 256)
            kqa_.append(
                np.concatenate([kau, qau], axis=2).reshape(80, 2 * N)
            )
            vv = v[b, h].reshape(NB, 128, D)
            vv = np.concatenate([vv, ones], axis=2)      # [NB, 128, 65]
            vp_.append(vv.transpose(1, 0, 2).reshape(128, NB * 65))
        in_maps.append({
            "kqa": np.ascontiguousarray(
                np.concatenate(kqa_, axis=1).astype(bf)
            ),
            "vp": np.ascontiguousarray(np.concatenate(vp_, axis=1).astype(bf)),
            "band2": np.ascontiguousarray(band2.astype(bf)),
        })
    return in_maps


def _reference_fallback(q, k, v, mask, group_size):
    """Pure-numpy fallback for inputs outside the compiled fast path
    (only reachable when the key-padding mask is not all-True)."""
    scale = D ** -0.5
    i = np.arange(q.shape[2])
    allowed = (i[None, :] // group_size) <= (i[:, None] // group_size)
    allowed &= i[None, :] >= i[:, None] - WIN
    allowed = allowed[None, :, :] & mask[:, None, :]
    bias = np.where(allowed, 0.0, -np.inf)[:, None, :, :]
    s = np.einsum("bhqd,bhkd->bhqk", q, k) * scale + bias
    s -= s.max(axis=-1, keepdims=True)
    p = np.exp(s)
    p /= p.sum(axis=-1, keepdims=True)
    return np.einsum("bhqk,bhkd->bhqd", p, v).astype(np.float32)


def kernel(q, k, v, mask, group_size):
    q = np.asarray(q, dtype=np.float32)
    k = np.asarray(k, dtype=np.float32)
    v = np.asarray(v, dtype=np.float32)
    mask = np.asarray(mask)
    if int(group_size) != G or q.shape != (B, H, N, D):
        return _reference_fallback(q, k, v, mask, int(group_size))
    if not mask.all():
        return _reference_fallback(q, k, v, mask, int(group_size))

    nc = _get_module()
    in_maps = _host_prep(q, k, v)
    res = run_bass_kernel_spmd(nc, in_maps, core_ids=list(range(NCORES)))
    out = np.empty((B, H, N, D), dtype=np.float32)
    for c in range(NCORES):
        o = np.asarray(res.results[c]["o"], dtype=np.float32)  # [2, 128, N]
        for hp in range(HPC):
            bh = HPC * c + hp
            out[bh // H, bh % H] = (o[hp, 0:64, :] / o[hp, 64:65, :]).T
    return out


if __name__ == "__main__":
    pass
